# revision 4
# baseline (speedup 1.0000x reference)
# Multi-head attention (B=4, L=2048, E=256, H=8) on 8 TRN2 NeuronCores.
#
# Sharding: core c handles batch b = c//2 and head group g = c%2 (heads
# 4g..4g+3).  Each core computes the partial output
#   sum_{h in group} softmax(q_h k_h^T / 16) v_h @ W_out[h]
# for its batch; the host adds the two head-group partials per batch.
#
# Per-core dataflow (all big matmuls in float32r, full PE rate at N>=256):
#   xT  = x^T                       (PE transpose, [256, 2048])
#   qT_h = wq_h^T x^T, kT_h likewise ([256, 2048], e on partitions)
#   v_h = x wv_h                    ([2048, 256], kj on partitions)
#   per 512-wide qi block, streaming over 16 kj tiles:
#     sT   = kT_h[:,kj]^T qT (PSUM [128kj, 512qi])
#     pT   = exp(sT / 16)    (ACT, PSUM->SBUF)
#     colacc += pT           (DVE running sum for the softmax denominator)
#     aoT  += v_h[kj]^T pT   (PSUM [128e, 512qi], accumulated over kj)
#   rowsum = colacc^T @ ones (PE, [128qi, 1]) ; recip = 1/rowsum (DVE)
#   out[qi] += (aoT^T @ wout_h) * recip   (per-partition scale on ACT)
# Scores never touch HBM; softmax normalization is applied after the
# output projection (row scaling commutes with right-multiplication).
# SBUF tiles feeding fp32r matmuls are declared float32r (the BIR
# verifier requires producers to round to fp32r); DVE/ACT consumers
# read them bitcast back to fp32.

import numpy as np

B, L, E, H = 4, 2048, 256, 8
HL = H // 2          # heads per core
LT = L // 128        # 16 row tiles
QB = 512             # qi block width
NQB = L // QB        # 4
KT = L // 128        # 16 kj tiles
SCALE = 1.0 / 16.0   # 1/sqrt(E)

_cache = {}


def _build_nc():
    import concourse.mybir as mybir
    from concourse import bacc
    from concourse.tile import TileContext
    from concourse.masks import make_identity

    F32 = mybir.dt.float32
    F32R = mybir.dt.float32r
    Exp = mybir.ActivationFunctionType.Exp
    Copy = mybir.ActivationFunctionType.Copy

    def f(ap):  # read a float32r tile as plain f32 (same bits)
        return ap.bitcast(F32)

    nc = bacc.Bacc(None, target_bir_lowering=False)

    x_d = nc.dram_tensor("x", [L, E], F32, kind="ExternalInput")
    wq_d = nc.dram_tensor("wq", [E, HL * E], F32, kind="ExternalInput")
    wk_d = nc.dram_tensor("wk", [E, HL * E], F32, kind="ExternalInput")
    wv_d = nc.dram_tensor("wv", [E, HL * E], F32, kind="ExternalInput")
    wout_d = nc.dram_tensor("wout", [HL * E, E], F32, kind="ExternalInput")
    out_d = nc.dram_tensor("out", [L, E], F32, kind="ExternalOutput")

    with TileContext(nc) as tc:
        with (
            tc.tile_pool(name="const", bufs=1) as cpool,
            tc.tile_pool(name="head", bufs=2) as hpool,
            tc.tile_pool(name="work", bufs=2) as wpool,
            tc.tile_pool(name="ps_s", bufs=2, space="PSUM") as ps_s,
            tc.tile_pool(name="ps_ao", bufs=2, space="PSUM") as ps_ao,
            tc.tile_pool(name="ps_proj", bufs=2, space="PSUM") as ps_proj,
            tc.tile_pool(name="ps_small", bufs=2, space="PSUM") as ps_small,
        ):
            ident = cpool.tile([128, 128], F32, name="ident")
            make_identity(nc, ident)
            ones = cpool.tile([128, 1], F32, name="ones")
            nc.gpsimd.memset(ones, 1.0)

            # ---- weights (resident, stored as float32r) ----
            wq_sb = [cpool.tile([128, HL * E], F32R, name=f"wq{i}") for i in range(2)]
            wk_sb = [cpool.tile([128, HL * E], F32R, name=f"wk{i}") for i in range(2)]
            wv_sb = [cpool.tile([128, HL * E], F32R, name=f"wv{i}") for i in range(2)]
            for i in range(2):
                nc.sync.dma_start(wq_sb[i], wq_d[i * 128:(i + 1) * 128, :].bitcast(F32R))
                nc.sync.dma_start(wk_sb[i], wk_d[i * 128:(i + 1) * 128, :].bitcast(F32R))
                nc.sync.dma_start(wv_sb[i], wv_d[i * 128:(i + 1) * 128, :].bitcast(F32R))
            wout_sb = [cpool.tile([128, E], F32R, name=f"wout{i}") for i in range(2 * HL)]
            for i in range(2 * HL):
                nc.sync.dma_start(wout_sb[i], wout_d[i * 128:(i + 1) * 128, :].bitcast(F32R))

            # ---- x load + transpose to xT [2][128, L] ----
            xT = [cpool.tile([128, L], F32R, name=f"xT{i}") for i in range(2)]
            for t in range(LT):
                xt = wpool.tile([128, E], F32, name="xt", tag="xt", bufs=3)
                nc.sync.dma_start(xt, x_d[t * 128:(t + 1) * 128, :])
                for eh in range(2):
                    pst = ps_small.tile([128, 128], F32, name="xtp", tag="small")
                    nc.tensor.transpose(pst, xt[:, eh * 128:(eh + 1) * 128], ident)
                    nc.vector.tensor_copy(xT[eh][:, t * 128:(t + 1) * 128], pst)

            out_acc = [cpool.tile([128, E], F32, name=f"oacc{t}") for t in range(LT)]

            for h in range(HL):
                # ---- projections for head h ----
                qT = [hpool.tile([128, L], F32R, name=f"qT{eh}", tag=f"qT{eh}")
                      for eh in range(2)]
                kT = [hpool.tile([128, L], F32R, name=f"kT{eh}", tag=f"kT{eh}")
                      for eh in range(2)]
                for eh in range(2):
                    for wsb, dst in ((wq_sb, qT), (wk_sb, kT)):
                        for nb in range(NQB):
                            ps = ps_proj.tile([128, QB], F32, name="projps", tag="pj")
                            for ih in range(2):
                                nc.tensor.matmul(
                                    ps,
                                    wsb[ih][:, h * E + eh * 128:h * E + (eh + 1) * 128],
                                    xT[ih][:, nb * QB:(nb + 1) * QB],
                                    start=(ih == 0), stop=(ih == 1),
                                )
                            nc.vector.tensor_copy(dst[eh][:, nb * QB:(nb + 1) * QB], ps)
                vt = [hpool.tile([128, E], F32R, name=f"v{t}", tag=f"v{t}")
                      for t in range(LT)]
                for t in range(LT):
                    ps = ps_proj.tile([128, E], F32, name="vps", tag="pj")
                    for ih in range(2):
                        nc.tensor.matmul(
                            ps,
                            xT[ih][:, t * 128:(t + 1) * 128],
                            wv_sb[ih][:, h * E:(h + 1) * E],
                            start=(ih == 0), stop=(ih == 1),
                        )
                    nc.vector.tensor_copy(vt[t], ps)

                # ---- attention, one 512-wide qi block at a time ----
                for qb in range(NQB):
                    colacc = wpool.tile([128, QB], F32, name="colacc", tag="colacc")
                    ao_ps = [ps_ao.tile([128, QB], F32, name=f"aops{eh}", tag="ao")
                             for eh in range(2)]
                    for t in range(KT):
                        s_ps = ps_s.tile([128, QB], F32, name="sps", tag="s")
                        for eh in range(2):
                            nc.tensor.matmul(
                                s_ps,
                                kT[eh][:, t * 128:(t + 1) * 128],
                                qT[eh][:, qb * QB:(qb + 1) * QB],
                                start=(eh == 0), stop=(eh == 1),
                            )
                        pt = wpool.tile([128, QB], F32R, name="pt", tag="pt", bufs=3)
                        nc.scalar.activation(pt, s_ps, Exp, scale=SCALE)
                        if t == 0:
                            nc.vector.tensor_copy(colacc, f(pt))
                        else:
                            nc.vector.tensor_add(colacc, colacc, f(pt))
                        for eh in range(2):
                            nc.tensor.matmul(
                                ao_ps[eh],
                                vt[t][:, eh * 128:(eh + 1) * 128],
                                pt,
                                start=(t == 0), stop=(t == KT - 1),
                            )
                    aoT = [wpool.tile([128, QB], F32R, name=f"aoT{eh}", tag=f"aoT{eh}")
                           for eh in range(2)]
                    for eh in range(2):
                        nc.vector.tensor_copy(aoT[eh], ao_ps[eh])
                    for j in range(QB // 128):
                        rs_ps = ps_small.tile([128, 1], F32, name="rsps", tag="small")
                        nc.tensor.matmul(rs_ps, colacc[:, j * 128:(j + 1) * 128],
                                         ones, start=True, stop=True)
                        recip = wpool.tile([128, 1], F32, name="recip", tag="recip",
                                           bufs=4)
                        nc.vector.reciprocal(recip, rs_ps)
                        pj_ps = ps_small.tile([128, E], F32, name="pjps", tag="small")
                        for eh in range(2):
                            nc.tensor.matmul(
                                pj_ps,
                                aoT[eh][:, j * 128:(j + 1) * 128],
                                wout_sb[2 * h + eh],
                                start=(eh == 0), stop=(eh == 1),
                            )
                        gt = qb * (QB // 128) + j
                        if h == 0:
                            nc.scalar.activation(out_acc[gt], pj_ps, Copy, scale=recip)
                        else:
                            tmp = wpool.tile([128, E], F32, name="ptmp", tag="ptmp")
                            nc.scalar.activation(tmp, pj_ps, Copy, scale=recip)
                            nc.vector.tensor_add(out_acc[gt], out_acc[gt], tmp)

            for t in range(LT):
                nc.sync.dma_start(out_d[t * 128:(t + 1) * 128, :], out_acc[t])

    nc.compile()
    return nc


def _get_nc():
    if "nc" not in _cache:
        _cache["nc"] = _build_nc()
    return _cache["nc"]


def _in_maps(x, W_qkv, W_out):
    x = np.ascontiguousarray(np.asarray(x, dtype=np.float32))
    W_qkv = np.ascontiguousarray(np.asarray(W_qkv, dtype=np.float32))
    W_out = np.ascontiguousarray(np.asarray(W_out, dtype=np.float32))
    maps = []
    for c in range(2 * B):
        b, g = c // 2, c % 2
        cw = HL * E * g  # 1024*g
        maps.append({
            "x": np.ascontiguousarray(x[b]),
            "wq": np.ascontiguousarray(W_qkv[:, cw:cw + HL * E]),
            "wk": np.ascontiguousarray(W_qkv[:, H * E + cw:H * E + cw + HL * E]),
            "wv": np.ascontiguousarray(W_qkv[:, 2 * H * E + cw:2 * H * E + cw + HL * E]),
            "wout": np.ascontiguousarray(W_out[cw:cw + HL * E, :]),
        })
    return maps


def kernel(x, W_qkv, W_out, _trace=False):
    from concourse.bass_utils import run_bass_kernel_spmd

    nc = _get_nc()
    maps = _in_maps(x, W_qkv, W_out)
    res = run_bass_kernel_spmd(nc, maps, core_ids=list(range(2 * B)),
                               trace=_trace)
    _cache["last_result"] = res
    outs = [m["out"] for m in res.results]
    full = np.stack([outs[2 * b] + outs[2 * b + 1] for b in range(B)])
    return full.astype(np.float32)
